# revision 23
# baseline (speedup 1.0000x reference)
"""Trainium2 Bass kernel for nn_NodeEdgeConv (GNN message passing).

Strategy (destination-sharded, matmul segment-sum, batched finish):
- Algebraic reduction: segment_sum(h[idx]*(v@W+b), idx)[n]
    = h[n] * (segment_sum(v, idx)[n] @ W + cnt[n]*b),
  so only the [E, 64] edge payloads need a device-side segment sum; all
  matmuls collapse to node-level GEMMs.
- Edges are sharded by DESTINATION node (node >> 10 -> core), so each core
  computes complete segment sums for its own 1024+1024 node shard; no
  collective needed.
- Host-side degree-sorted slotting: per (core, side), nodes are sorted by
  edge count and assigned to (block, partition) slots; fp8 edge payloads
  are laid out in tiles of [128 tokens, 64] where partition p always
  belongs to node slot p of the current block. The device-side segment sum
  is then just `psum += tile` -- a matmul with a constant fp8 identity
  stationary. Zero per-edge index processing on device.
- Finish (Linear+LayerNorm+Linear residual) runs batched per side in
  transposed orientation [D, 1024]: all weight matmuls use constant
  stationaries (bias/cnt folded in via a 65-row message weight; LayerNorm
  gamma/beta and final bias folded into W2/embeddings on the host). Only
  LayerNorm statistics round-trip through node orientation via PE
  transposes. Outputs are written transposed and unscrambled on the host.
"""

import numpy as np
import ml_dtypes

import concourse.bass as bass
import concourse.bacc as bacc
import concourse.mybir as mybir
import concourse.tile as tile

F32 = mybir.dt.float32
BF16 = mybir.dt.bfloat16
F8 = mybir.dt.float8e4
BF16_NP = ml_dtypes.bfloat16
F8_NP = ml_dtypes.float8_e4m3


class Cfg:
    def __init__(self):
        self.N = 8192          # nodes per side
        self.E = 524288        # edges per type
        self.D = 128
        self.M = 64
        self.C = 8             # cores
        self.NSH = self.N // self.C      # 1024 nodes per core per side
        self.NB = self.NSH // 128        # 8 blocks per side
        self.CH = 384          # tiles per DMA chunk (3 MB fp8)


# ---------------- host-side schedule + layout ----------------

def host_prep(inputs, cfg):
    """Shard edges by destination, degree-sort nodes into (block, partition)
    slots, lay out payload tiles, fold biases. Returns
    (in_maps, sched, TOT, perms)."""
    C, NSH, NB, M, CH, D = cfg.C, cfg.NSH, cfg.NB, cfg.M, cfg.CH, cfg.D

    sides = [
        (np.asarray(inputs["e_s2d_dst"]), np.asarray(inputs["v_s2d"], np.float32)),
        (np.asarray(inputs["e_d2s_dst"]), np.asarray(inputs["v_d2s"], np.float32)),
    ]

    percore = [[None] * 2 for _ in range(C)]
    for s, (idx_all, v_all) in enumerate(sides):
        core_of = idx_all // NSH
        for c in range(C):
            esel = np.flatnonzero(core_of == c)
            loc = idx_all[esel] - c * NSH
            cnt = np.bincount(loc, minlength=NSH)
            order = np.argsort(-cnt, kind="stable")
            percore[c][s] = (esel, loc, cnt, order)

    # SPMD envelope: per-block tile count = max over cores of block max count
    T = np.zeros((2, NB), np.int64)
    for s in range(2):
        for c in range(C):
            cnt, order = percore[c][s][2], percore[c][s][3]
            sc = cnt[order]
            for b in range(NB):
                T[s][b] = max(T[s][b], sc[128 * b])
    T = np.maximum(T, 1)
    sched = tuple(int(x) for x in T.reshape(-1))
    block_off = np.zeros((2, NB), np.int64)
    off = 0
    for s in range(2):
        for b in range(NB):
            block_off[s][b] = off
            off += T[s][b]
    TILES = off
    TOT = TILES

    semb = np.asarray(inputs["src_embed"], np.float32)
    demb = np.asarray(inputs["dst_embed"], np.float32)
    emb_by_side = [demb, semb]     # side 0 (s2d) -> dst nodes, side 1 -> src

    def f32(k):
        return np.asarray(inputs[k], np.float32)

    # side-stacked folded weights (side 0 = "col"/dst, side 1 = "row"/src)
    Wside = np.stack([f32("W_dst"), f32("W_src")]).astype(BF16_NP)
    Wmh = np.stack([
        np.vstack([f32("W_sm"), f32("b_sm")[None]]),
        np.vstack([f32("W_dm"), f32("b_dm")[None]]),
    ]).astype(BF16_NP)                                   # [2, M+1, D]
    W1 = np.stack([f32("col_W1"), f32("row_W1")]).astype(BF16_NP)
    W2g = np.stack([
        f32("col_g")[:, None] * f32("col_W2"),
        f32("row_g")[:, None] * f32("row_W2"),
    ]).astype(BF16_NP)
    b2p = [f32("col_beta") @ f32("col_W2") + f32("col_b2"),
           f32("row_beta") @ f32("row_W2") + f32("row_b2")]
    bcols = np.stack([f32("b_dst"), f32("b_src"),
                      f32("col_b1"), f32("row_b1")], axis=1)   # [128, 4]
    identb = np.eye(128, dtype=F8_NP)
    identb16 = np.eye(128, dtype=BF16_NP)

    common = {"identb": identb, "identb16": identb16, "Wside_b": Wside,
              "Wmh_b": Wmh, "W1_b": W1, "W2g_b": W2g,
              "bcols": np.ascontiguousarray(bcols)}

    in_maps = []
    perms = []
    for c in range(C):
        vtiles = np.zeros((TOT, 128, M), F8_NP)
        cntR = np.zeros((2, 1, NB, 128), BF16_NP)
        embT16 = np.zeros((2, D, NSH), BF16_NP)
        embTb2 = np.zeros((2, D, NSH), np.float32)
        ords = []
        for s, (idx_all, v_all) in enumerate(sides):
            esel, loc, cnt, order = percore[c][s]
            ords.append(order)
            pos = np.empty(NSH, np.int64)
            pos[order] = np.arange(NSH)
            eorder = np.argsort(loc, kind="stable")
            starts = np.zeros(NSH + 1, np.int64)
            np.cumsum(cnt, out=starts[1:])
            rank = np.arange(len(eorder)) - starts[loc[eorder]]
            p_of = pos[loc[eorder]]
            tile_of = block_off[s][p_of // 128] + rank
            flat = tile_of * 128 + (p_of % 128)
            vtiles.reshape(-1, M)[flat] = v_all[esel[eorder]].astype(F8_NP)
            cntR[s, 0] = cnt[order].astype(BF16_NP).reshape(NB, 128)
            embT = emb_by_side[s][c * NSH:(c + 1) * NSH][order].T
            embT16[s] = embT.astype(BF16_NP)
            embTb2[s] = embT + b2p[s][:, None]
        m = dict(common)
        m["vhw"] = np.ascontiguousarray(np.concatenate(
            [vtiles[a:a + n].transpose(1, 0, 2).reshape(128, n * M)
             for a, n in chunk_table(TILES)], axis=1))
        m["cntR"] = cntR
        m["embT16"] = embT16
        m["embTb2"] = embTb2
        in_maps.append(m)
        perms.append(ords)
    return in_maps, sched, TOT, perms


def chunk_table(tiles):
    """Ramped chunk sizes: small leading chunks so the PE starts early,
    then large chunks for DMA efficiency."""
    sizes = []
    rem = tiles
    for r in [32, 64, 128, 256]:
        n = min(r, rem)
        if n:
            sizes.append(n)
            rem -= n
    while rem:
        n = min(384, rem)
        sizes.append(n)
        rem -= n
    out = []
    off = 0
    for n in sizes:
        out.append((off, n))
        off += n
    return out


# ---------------- device kernel ----------------

def build_kernel(cfg, sched, TOT, reps=1, mode="full"):
    import contextlib
    C, D, M, NSH, NB, CH = cfg.C, cfg.D, cfg.M, cfg.NSH, cfg.NB, cfg.CH
    T = np.asarray(sched, np.int64).reshape(2, NB)
    nc = bacc.Bacc("TRN2", target_bir_lowering=False, debug=False, num_devices=C)

    vhw = nc.dram_tensor("vhw", [128, TOT * M], F8, kind="ExternalInput")
    identb_d = nc.dram_tensor("identb", [128, 128], F8, kind="ExternalInput")
    identb16_d = nc.dram_tensor("identb16", [128, 128], BF16, kind="ExternalInput")
    Wside_d = nc.dram_tensor("Wside_b", [2, D, D], BF16, kind="ExternalInput")
    Wmh_d = nc.dram_tensor("Wmh_b", [2, M + 1, D], BF16, kind="ExternalInput")
    W1_d = nc.dram_tensor("W1_b", [2, D, D], BF16, kind="ExternalInput")
    W2g_d = nc.dram_tensor("W2g_b", [2, D, D], BF16, kind="ExternalInput")
    bcols_d = nc.dram_tensor("bcols", [128, 4], F32, kind="ExternalInput")
    cntR_d = nc.dram_tensor("cntR", [2, 1, NB, 128], BF16, kind="ExternalInput")
    embT16_d = nc.dram_tensor("embT16", [2, D, NSH], BF16, kind="ExternalInput")
    embTb2_d = nc.dram_tensor("embTb2", [2, D, NSH], F32, kind="ExternalInput")
    rowo = nc.dram_tensor("rowo", [D, NSH], F32, kind="ExternalOutput")
    colo = nc.dram_tensor("colo", [D, NSH], F32, kind="ExternalOutput")

    with tile.TileContext(nc) as tc:
        with (
            tc.tile_pool(name="const", bufs=1) as const,
            tc.tile_pool(name="io", bufs=4) as io,
            tc.tile_pool(name="fin", bufs=3) as fin,
            tc.tile_pool(name="psa", bufs=2, space="PSUM") as psa,
            tc.tile_pool(name="psb", bufs=1, space="PSUM") as psb,
            tc.tile_pool(name="psw", bufs=1, space="PSUM") as psw,
        ):
            identb = const.tile([128, 128], F8)
            nc.sync.dma_start(identb[:], identb_d.ap())
            identb16 = const.tile([128, 128], BF16)
            nc.sync.dma_start(identb16[:], identb16_d.ap())
            eps = const.tile([128, 1], F32)
            nc.vector.memset(eps[:], 1e-5)

            def load2(dram, shp, tag):
                ts = []
                for s in range(2):
                    t = const.tile(shp, BF16, tag=f"{tag}{s}")
                    nc.sync.dma_start(t[:], dram.ap()[s])
                    ts.append(t)
                return ts

            Wside_sb = load2(Wside_d, [D, D], "Wside")
            Wmh_sb = load2(Wmh_d, [M + 1, D], "Wmh")
            W1_sb = load2(W1_d, [D, D], "W1")
            W2g_sb = load2(W2g_d, [D, D], "W2g")
            bcols = const.tile([128, 4], F32)
            nc.sync.dma_start(bcols[:], bcols_d.ap())
            embT16_sb = const.tile([128, 2, NSH], BF16)
            nc.sync.dma_start(embT16_sb[:], embT16_d.ap().rearrange(
                "s d n -> d s n"))
            embTb2_sb = const.tile([128, 2, NSH], F32)
            nc.sync.dma_start(embTb2_sb[:], embTb2_d.ap().rearrange(
                "s d n -> d s n"))
            # AT tiles: rows 0..63 written per side per rep; row 64 = cnt
            AT_sb = []
            for s in range(2):
                t = const.tile([M + 1, NB, 128], BF16, tag=f"AT{s}")
                nc.sync.dma_start(t[M:M + 1, :, :], cntR_d.ap()[s])
                AT_sb.append(t)

            rep_ctx = tc.For_i(0, reps) if reps > 1 else contextlib.nullcontext()
            with rep_ctx:
                run_body(nc, tc, cfg, T, io, fin, psa, psb, psw,
                         vhw, identb, identb16, eps, Wside_sb, Wmh_sb,
                         W1_sb, W2g_sb, bcols, embT16_sb, embTb2_sb,
                         AT_sb, colo, rowo, mode)

    nc.compile()
    return nc


def run_body(nc, tc, cfg, T, io, fin, psa, psb, psw, vhw, identb, identb16,
             eps, Wside_sb, Wmh_sb, W1_sb, W2g_sb, bcols, embT16_sb,
             embTb2_sb, AT_sb, colo, rowo, mode="full"):
    D, M, NB, CH = cfg.D, cfg.M, cfg.NB, cfg.CH
    NW = NB * 128          # nodes per side (1024)
    ntiles_all = int(T.sum())
    chunks = chunk_table(ntiles_all)
    tile2chunk = np.empty(ntiles_all, np.int64)
    for ci, (a, n) in enumerate(chunks):
        tile2chunk[a:a + n] = ci
    cur_chunk = [None, -1]

    def chunk_for(tidx):
        cidx = int(tile2chunk[tidx])
        if cur_chunk[1] != cidx:
            a, n = chunks[cidx]
            t = io.tile([128, CH, M], F8, tag="vchunk")
            nc.sync.dma_start(
                t[:, :n, :],
                vhw.ap()[:, a * M:(a + n) * M].rearrange(
                    "p (t m) -> p t m", m=M))
            cur_chunk[0], cur_chunk[1] = t, cidx
        return cur_chunk[0]

    if mode == "empty":
        z = fin.tile([128, 1], F32, tag="z")
        nc.vector.memset(z[:], 0.0)
        return

    if mode == "dma":
        for tidx in range(ntiles_all):
            chunk_for(tidx)
        return

    tidx = 0
    for s in range(2):
        out_d = colo if s == 0 else rowo

        # ---- segment sums for all 8 blocks into one PSUM bank ----
        A_w = psa.tile([128, NB * M], F32, tag="A")
        for b in range(NB):
            Tb = int(T[s][b])
            for t in range(Tb):
                ck = chunk_for(tidx)
                nc.tensor.matmul(
                    A_w[:, b * M:(b + 1) * M], lhsT=identb[:],
                    rhs=ck[:, tidx - chunks[int(tile2chunk[tidx])][0], :],
                    start=(t == 0), stop=(t == Tb - 1),
                    skip_group_check=True)
                tidx += 1

        A_sb = fin.tile([128, NB, M], BF16, tag="Asb")
        nc.vector.tensor_copy(
            A_sb[:].rearrange("p b m -> p (b m)"), A_w[:])

        if mode == "main":
            af = fin.tile([128, NB * M], F32, tag="Af")
            nc.vector.tensor_copy(af[:], A_w[:])
            nc.sync.dma_start(out_d.ap()[:, :NB * M // 2],
                              af[:, :NB * M // 2])
            continue

        # ---- A^T via PE transposes -> AT rows 0..63 (row 64 = cnt) ----
        at_w = psb.tile([M, NB * 128], BF16, tag="tr1")
        for b in range(NB):
            nc.tensor.transpose(
                at_w[:, b * 128:(b + 1) * 128], A_sb[:, b, :], identb16[:])
        nc.scalar.activation(
            AT_sb[s][:M, :, :].rearrange("m b n -> m (b n)"), at_w[:],
            func=mybir.ActivationFunctionType.Copy)

        # ---- h^T = (emb @ Wside)^T ; S^T = (A @ Wm + cnt*bm)^T ----
        h_ps = psw.tile([128, 2, 512], F32, tag="mm1")
        s_ps = psw.tile([128, 2, 512], F32, tag="mm2")
        for j in range(2):
            nc.tensor.matmul(
                h_ps[:, j, :], lhsT=Wside_sb[s][:],
                rhs=embT16_sb[:, s, j * 512:(j + 1) * 512])
            nc.tensor.matmul(
                s_ps[:, j, :], lhsT=Wmh_sb[s][:],
                rhs=AT_sb[s][:].rearrange("m b n -> m (b n)")[
                    :, j * 512:(j + 1) * 512])
        h2 = fin.tile([128, NW], F32, tag="h2")
        nc.vector.tensor_scalar_add(
            h2[:], h_ps[:].rearrange("p j n -> p (j n)"),
            scalar1=bcols[:, s:s + 1])
        u_sb = fin.tile([128, NW], BF16, tag="u")
        nc.vector.tensor_mul(
            u_sb[:], h2[:], s_ps[:].rearrange("p j n -> p (j n)"))

        # ---- t1^T = (u @ W1)^T + b1 ----
        t1_ps = psw.tile([128, 2, 512], F32, tag="mm1")
        for j in range(2):
            nc.tensor.matmul(t1_ps[:, j, :], lhsT=W1_sb[s][:],
                             rhs=u_sb[:, j * 512:(j + 1) * 512])
        t1T = fin.tile([128, NW], BF16, tag="t1T")
        nc.vector.tensor_scalar_add(
            t1T[:], t1_ps[:].rearrange("p j n -> p (j n)"),
            scalar1=bcols[:, 2 + s:3 + s])

        # ---- LayerNorm stats in node orientation ----
        t1w = psb.tile([128, NW], BF16, tag="tr2")
        for b in range(NB):
            nc.tensor.transpose(
                t1w[:, b * 128:(b + 1) * 128],
                t1T[:, b * 128:(b + 1) * 128], identb16[:])
        t1n = fin.tile([128, NB, 128], BF16, tag="t1n")
        nc.scalar.activation(
            t1n[:].rearrange("p b d -> p (b d)"), t1w[:],
            func=mybir.ActivationFunctionType.Copy)
        sum_t = fin.tile([128, NB], F32, tag="sum")
        nc.vector.tensor_reduce(sum_t[:], t1n[:], axis=mybir.AxisListType.X,
                                op=mybir.AluOpType.add)
        sq = fin.tile([128, NB, 128], BF16, tag="sq")
        nc.scalar.activation(sq[:], t1n[:],
                             func=mybir.ActivationFunctionType.Square)
        ssq = fin.tile([128, NB], F32, tag="ssq")
        nc.vector.tensor_reduce(ssq[:], sq[:], axis=mybir.AxisListType.X,
                                op=mybir.AluOpType.add)
        mu = fin.tile([128, NB], F32, tag="mu")
        nc.vector.tensor_scalar_mul(mu[:], in0=sum_t[:], scalar1=1.0 / D)
        var = fin.tile([128, NB], F32, tag="var")
        nc.vector.tensor_scalar_mul(var[:], in0=ssq[:], scalar1=1.0 / D)
        m2 = fin.tile([128, NB], F32, tag="m2")
        nc.vector.tensor_mul(m2[:], mu[:], mu[:])
        nc.vector.tensor_sub(var[:], var[:], m2[:])
        rstd = fin.tile([128, NB], F32, tag="rstd")
        nc.scalar.activation(rstd[:], var[:],
                             func=mybir.ActivationFunctionType.Sqrt,
                             bias=eps[:], scale=1.0)
        nc.vector.reciprocal(rstd[:], rstd[:])
        that = fin.tile([128, NB, 128], BF16, tag="that")
        for b in range(NB):
            nc.vector.tensor_scalar(
                that[:, b, :], in0=t1n[:, b, :],
                scalar1=mu[:, b:b + 1], scalar2=rstd[:, b:b + 1],
                op0=mybir.AluOpType.subtract, op1=mybir.AluOpType.mult)

        # ---- t2^T = (that @ W2g)^T ; out^T = t2^T + embT + b2p ----
        tt_w = psb.tile([128, NW], BF16, tag="tr2")
        for b in range(NB):
            nc.tensor.transpose(
                tt_w[:, b * 128:(b + 1) * 128], that[:, b, :], identb16[:])
        tT = fin.tile([128, NW], BF16, tag="tT")
        nc.scalar.activation(tT[:], tt_w[:],
                             func=mybir.ActivationFunctionType.Copy)
        t2_ps = psw.tile([128, 2, 512], F32, tag="mm2")
        for j in range(2):
            nc.tensor.matmul(t2_ps[:, j, :], lhsT=W2g_sb[s][:],
                             rhs=tT[:, j * 512:(j + 1) * 512])
        ot = fin.tile([128, NW], F32, tag="ot")
        nc.vector.tensor_add(
            ot[:], t2_ps[:].rearrange("p j n -> p (j n)"),
            embTb2_sb[:, s, :])
        nc.sync.dma_start(out_d.ap(), ot[:])


def assemble(results, perms, cfg):
    NSH = cfg.NSH
    row = np.empty((cfg.N, cfg.D), np.float32)
    col = np.empty((cfg.N, cfg.D), np.float32)
    for c, r in enumerate(results):
        ord_s2d, ord_d2s = perms[c]
        col[c * NSH + ord_s2d] = r["colo"].T
        row[c * NSH + ord_d2s] = r["rowo"].T
    return row, col


# ---------------- graded entry point ----------------

_CACHE = {}


def kernel(**inputs):
    cfg = Cfg()
    in_maps, sched, TOT, perms = host_prep(inputs, cfg)
    key = (sched, TOT)
    if key not in _CACHE:
        _CACHE[key] = build_kernel(cfg, sched, TOT)
    nc = _CACHE[key]
    from concourse.bass_utils import run_bass_kernel_spmd
    res = run_bass_kernel_spmd(nc, in_maps, core_ids=list(range(cfg.C)))
    return assemble(res.results, perms, cfg)


# revision 24
# speedup vs baseline: 1.2132x; 1.2132x over previous
"""Trainium2 Bass kernel for nn_NodeEdgeConv (GNN message passing).

Strategy (destination-sharded, matmul segment-sum, batched finish):
- Algebraic reduction: segment_sum(h[idx]*(v@W+b), idx)[n]
    = h[n] * (segment_sum(v, idx)[n] @ W + cnt[n]*b),
  so only the [E, 64] edge payloads need a device-side segment sum; all
  matmuls collapse to node-level GEMMs.
- Edges are sharded by DESTINATION node (node >> 10 -> core), so each core
  computes complete segment sums for its own 1024+1024 node shard; no
  collective needed.
- Host-side degree-sorted slotting: per (core, side), nodes are sorted by
  edge count and assigned to (block, partition) slots; fp8 edge payloads
  are laid out in tiles of [128 tokens, 64] where partition p always
  belongs to node slot p of the current block. The device-side segment sum
  is then just `psum += tile` -- a matmul with a constant fp8 identity
  stationary. Zero per-edge index processing on device.
- Finish (Linear+LayerNorm+Linear residual) runs batched per side in
  transposed orientation [D, 1024]: all weight matmuls use constant
  stationaries (bias/cnt folded in via a 65-row message weight; LayerNorm
  gamma/beta and final bias folded into W2/embeddings on the host). Only
  LayerNorm statistics round-trip through node orientation via PE
  transposes. Outputs are written transposed and unscrambled on the host.
"""

import numpy as np
import ml_dtypes

import concourse.bass as bass
import concourse.bacc as bacc
import concourse.mybir as mybir
import concourse.tile as tile

F32 = mybir.dt.float32
BF16 = mybir.dt.bfloat16
F8 = mybir.dt.float8e4
BF16_NP = ml_dtypes.bfloat16
F8_NP = ml_dtypes.float8_e4m3


class Cfg:
    def __init__(self):
        self.N = 8192          # nodes per side
        self.E = 524288        # edges per type
        self.D = 128
        self.M = 64
        self.C = 8             # cores
        self.NSH = self.N // self.C      # 1024 nodes per core per side
        self.NB = self.NSH // 128        # 8 blocks per side
        self.CH = 384          # tiles per DMA chunk (3 MB fp8)


# ---------------- host-side schedule + layout ----------------

def host_prep(inputs, cfg):
    """Shard edges by destination, degree-sort nodes into (block, partition)
    slots, lay out payload tiles, fold biases. Returns
    (in_maps, sched, TOT, perms)."""
    C, NSH, NB, M, CH, D = cfg.C, cfg.NSH, cfg.NB, cfg.M, cfg.CH, cfg.D

    sides = [
        (np.asarray(inputs["e_s2d_dst"]), np.asarray(inputs["v_s2d"], np.float32)),
        (np.asarray(inputs["e_d2s_dst"]), np.asarray(inputs["v_d2s"], np.float32)),
    ]

    percore = [[None] * 2 for _ in range(C)]
    for s, (idx_all, v_all) in enumerate(sides):
        core_of = idx_all // NSH
        for c in range(C):
            esel = np.flatnonzero(core_of == c)
            loc = idx_all[esel] - c * NSH
            cnt = np.bincount(loc, minlength=NSH)
            order = np.argsort(-cnt, kind="stable")
            percore[c][s] = (esel, loc, cnt, order)

    # SPMD envelope: per-block tile count = max over cores of block max count
    T = np.zeros((2, NB), np.int64)
    for s in range(2):
        for c in range(C):
            cnt, order = percore[c][s][2], percore[c][s][3]
            sc = cnt[order]
            for b in range(NB):
                T[s][b] = max(T[s][b], sc[128 * b])
    T = np.maximum(T, 1)
    sched = tuple(int(x) for x in T.reshape(-1))
    block_off = np.zeros((2, NB), np.int64)
    off = 0
    for s in range(2):
        for b in range(NB):
            block_off[s][b] = off
            off += T[s][b]
    TILES = off
    TOT = TILES

    semb = np.asarray(inputs["src_embed"], np.float32)
    demb = np.asarray(inputs["dst_embed"], np.float32)
    emb_by_side = [demb, semb]     # side 0 (s2d) -> dst nodes, side 1 -> src

    def f32(k):
        return np.asarray(inputs[k], np.float32)

    # side-stacked folded weights (side 0 = "col"/dst, side 1 = "row"/src)
    Wside = np.stack([f32("W_dst"), f32("W_src")]).astype(BF16_NP)
    Wmh = np.stack([
        np.vstack([f32("W_sm"), f32("b_sm")[None]]),
        np.vstack([f32("W_dm"), f32("b_dm")[None]]),
    ]).astype(BF16_NP)                                   # [2, M+1, D]
    W1 = np.stack([f32("col_W1"), f32("row_W1")]).astype(BF16_NP)
    W2g = np.stack([
        f32("col_g")[:, None] * f32("col_W2"),
        f32("row_g")[:, None] * f32("row_W2"),
    ]).astype(BF16_NP)
    b2p = [f32("col_beta") @ f32("col_W2") + f32("col_b2"),
           f32("row_beta") @ f32("row_W2") + f32("row_b2")]
    bcols = np.stack([f32("b_dst"), f32("b_src"),
                      f32("col_b1"), f32("row_b1")], axis=1)   # [128, 4]
    identb = np.eye(128, dtype=F8_NP)
    identb16 = np.eye(128, dtype=BF16_NP)

    common = {"identb": identb, "identb16": identb16, "Wside_b": Wside,
              "Wmh_b": Wmh, "W1_b": W1, "W2g_b": W2g,
              "bcols": np.ascontiguousarray(bcols)}

    in_maps = []
    perms = []
    for c in range(C):
        vtiles = np.zeros((TOT, 128, M), F8_NP)
        cntR = np.zeros((2, 1, NB, 128), BF16_NP)
        embT16 = np.zeros((2, D, NSH), BF16_NP)
        embTb2 = np.zeros((2, D, NSH), np.float32)
        ords = []
        for s, (idx_all, v_all) in enumerate(sides):
            esel, loc, cnt, order = percore[c][s]
            ords.append(order)
            pos = np.empty(NSH, np.int64)
            pos[order] = np.arange(NSH)
            eorder = np.argsort(loc, kind="stable")
            starts = np.zeros(NSH + 1, np.int64)
            np.cumsum(cnt, out=starts[1:])
            rank = np.arange(len(eorder)) - starts[loc[eorder]]
            p_of = pos[loc[eorder]]
            tile_of = block_off[s][p_of // 128] + rank
            flat = tile_of * 128 + (p_of % 128)
            vtiles.reshape(-1, M)[flat] = v_all[esel[eorder]].astype(F8_NP)
            cntR[s, 0] = cnt[order].astype(BF16_NP).reshape(NB, 128)
            embT = emb_by_side[s][c * NSH:(c + 1) * NSH][order].T
            embT16[s] = embT.astype(BF16_NP)
            embTb2[s] = embT + b2p[s][:, None]
        m = dict(common)
        m["vhw"] = np.ascontiguousarray(np.concatenate(
            [vtiles[a:a + n].transpose(1, 0, 2).reshape(128, n * M)
             for a, n in chunk_table(TILES)], axis=1))
        m["cntR"] = cntR
        m["embT16"] = embT16
        m["embTb2"] = embTb2
        in_maps.append(m)
        perms.append(ords)
    return in_maps, sched, TOT, perms


def chunk_table(tiles):
    """Ramped chunk sizes: small leading chunks so the PE starts early,
    then large chunks for DMA efficiency."""
    sizes = []
    rem = tiles
    for r in [32, 64, 128, 256]:
        n = min(r, rem)
        if n:
            sizes.append(n)
            rem -= n
    while rem:
        n = min(384, rem)
        sizes.append(n)
        rem -= n
    out = []
    off = 0
    for n in sizes:
        out.append((off, n))
        off += n
    return out


# ---------------- device kernel ----------------

def build_kernel(cfg, sched, TOT, reps=1, mode="full"):
    import contextlib
    C, D, M, NSH, NB, CH = cfg.C, cfg.D, cfg.M, cfg.NSH, cfg.NB, cfg.CH
    T = np.asarray(sched, np.int64).reshape(2, NB)
    nc = bacc.Bacc("TRN2", target_bir_lowering=False, debug=False, num_devices=C)

    vhw = nc.dram_tensor("vhw", [128, TOT * M], F8, kind="ExternalInput")
    identb_d = nc.dram_tensor("identb", [128, 128], F8, kind="ExternalInput")
    identb16_d = nc.dram_tensor("identb16", [128, 128], BF16, kind="ExternalInput")
    Wside_d = nc.dram_tensor("Wside_b", [2, D, D], BF16, kind="ExternalInput")
    Wmh_d = nc.dram_tensor("Wmh_b", [2, M + 1, D], BF16, kind="ExternalInput")
    W1_d = nc.dram_tensor("W1_b", [2, D, D], BF16, kind="ExternalInput")
    W2g_d = nc.dram_tensor("W2g_b", [2, D, D], BF16, kind="ExternalInput")
    bcols_d = nc.dram_tensor("bcols", [128, 4], F32, kind="ExternalInput")
    cntR_d = nc.dram_tensor("cntR", [2, 1, NB, 128], BF16, kind="ExternalInput")
    embT16_d = nc.dram_tensor("embT16", [2, D, NSH], BF16, kind="ExternalInput")
    embTb2_d = nc.dram_tensor("embTb2", [2, D, NSH], F32, kind="ExternalInput")
    rowo = nc.dram_tensor("rowo", [D, NSH], F32, kind="ExternalOutput")
    colo = nc.dram_tensor("colo", [D, NSH], F32, kind="ExternalOutput")

    with tile.TileContext(nc) as tc:
        with (
            tc.tile_pool(name="const", bufs=1) as const,
            tc.tile_pool(name="io", bufs=3) as io,
            tc.tile_pool(name="fin", bufs=2) as fin,
            tc.tile_pool(name="psa", bufs=2, space="PSUM") as psa,
            tc.tile_pool(name="psb", bufs=1, space="PSUM") as psb,
            tc.tile_pool(name="psw", bufs=1, space="PSUM") as psw,
        ):
            identb = const.tile([128, 128], F8)
            nc.sync.dma_start(identb[:], identb_d.ap())
            identb16 = const.tile([128, 128], BF16)
            nc.sync.dma_start(identb16[:], identb16_d.ap())
            eps = const.tile([128, 1], F32)
            nc.vector.memset(eps[:], 1e-5)

            def load2(dram, shp, tag):
                ts = []
                for s in range(2):
                    t = const.tile(shp, BF16, tag=f"{tag}{s}")
                    nc.sync.dma_start(t[:], dram.ap()[s])
                    ts.append(t)
                return ts

            Wside_sb = load2(Wside_d, [D, D], "Wside")
            Wmh_sb = load2(Wmh_d, [M + 1, D], "Wmh")
            W1_sb = load2(W1_d, [D, D], "W1")
            W2g_sb = load2(W2g_d, [D, D], "W2g")
            bcols = const.tile([128, 4], F32)
            nc.sync.dma_start(bcols[:], bcols_d.ap())
            embT16_sb = const.tile([128, 2, NSH], BF16)
            nc.sync.dma_start(embT16_sb[:], embT16_d.ap().rearrange(
                "s d n -> d s n"))
            embTb2_sb = const.tile([128, 2, NSH], F32)
            nc.sync.dma_start(embTb2_sb[:], embTb2_d.ap().rearrange(
                "s d n -> d s n"))
            # AT tiles: rows 0..63 written per side per rep; row 64 = cnt
            AT_sb = []
            for s in range(2):
                t = const.tile([M + 1, NB, 128], BF16, tag=f"AT{s}")
                nc.sync.dma_start(t[M:M + 1, :, :], cntR_d.ap()[s])
                AT_sb.append(t)

            rep_ctx = tc.For_i(0, reps) if reps > 1 else contextlib.nullcontext()
            with rep_ctx:
                run_body(nc, tc, cfg, T, io, fin, psa, psb, psw,
                         vhw, identb, identb16, eps, Wside_sb, Wmh_sb,
                         W1_sb, W2g_sb, bcols, embT16_sb, embTb2_sb,
                         AT_sb, colo, rowo, mode)

    nc.compile()
    return nc


def run_body(nc, tc, cfg, T, io, fin, psa, psb, psw, vhw, identb, identb16,
             eps, Wside_sb, Wmh_sb, W1_sb, W2g_sb, bcols, embT16_sb,
             embTb2_sb, AT_sb, colo, rowo, mode="full"):
    D, M, NB, CH = cfg.D, cfg.M, cfg.NB, cfg.CH
    NW = NB * 128          # nodes per side (1024)
    ntiles_all = int(T.sum())
    chunks = chunk_table(ntiles_all)
    tile2chunk = np.empty(ntiles_all, np.int64)
    for ci, (a, n) in enumerate(chunks):
        tile2chunk[a:a + n] = ci
    cur_chunk = [None, -1]

    def chunk_for(tidx):
        cidx = int(tile2chunk[tidx])
        if cur_chunk[1] != cidx:
            a, n = chunks[cidx]
            t = io.tile([128, CH, M], F8, tag="vchunk")
            nc.sync.dma_start(
                t[:, :n, :],
                vhw.ap()[:, a * M:(a + n) * M].rearrange(
                    "p (t m) -> p t m", m=M))
            cur_chunk[0], cur_chunk[1] = t, cidx
        return cur_chunk[0]

    if mode == "empty":
        z = fin.tile([128, 1], F32, tag="z")
        nc.vector.memset(z[:], 0.0)
        return

    if mode == "dma":
        for tidx in range(ntiles_all):
            chunk_for(tidx)
        return

    tidx = 0
    for s in range(2):
        out_d = colo if s == 0 else rowo

        # ---- segment sums for all 8 blocks into one PSUM bank ----
        A_w = psa.tile([128, NB * M], F32, tag="A")
        for b in range(NB):
            Tb = int(T[s][b])
            for t in range(Tb):
                ck = chunk_for(tidx)
                nc.tensor.matmul(
                    A_w[:, b * M:(b + 1) * M], lhsT=identb[:],
                    rhs=ck[:, tidx - chunks[int(tile2chunk[tidx])][0], :],
                    start=(t == 0), stop=(t == Tb - 1),
                    skip_group_check=True)
                tidx += 1

        A_sb = fin.tile([128, NB, M], BF16, tag="Asb")
        nc.vector.tensor_copy(
            A_sb[:].rearrange("p b m -> p (b m)"), A_w[:])

        if mode == "main":
            af = fin.tile([128, NB * M], F32, tag="Af")
            nc.vector.tensor_copy(af[:], A_w[:])
            nc.sync.dma_start(out_d.ap()[:, :NB * M // 2],
                              af[:, :NB * M // 2])
            continue

        # ---- A^T via PE transposes -> AT rows 0..63 (row 64 = cnt) ----
        at_w = psb.tile([M, NB * 128], BF16, tag="tr1")
        for b in range(NB):
            nc.tensor.transpose(
                at_w[:, b * 128:(b + 1) * 128], A_sb[:, b, :], identb16[:])
        nc.scalar.activation(
            AT_sb[s][:M, :, :].rearrange("m b n -> m (b n)"), at_w[:],
            func=mybir.ActivationFunctionType.Copy)

        # ---- h^T = (emb @ Wside)^T ; S^T = (A @ Wm + cnt*bm)^T ----
        h_ps = psw.tile([128, 2, 512], F32, tag="mm1")
        s_ps = psw.tile([128, 2, 512], F32, tag="mm2")
        for j in range(2):
            nc.tensor.matmul(
                h_ps[:, j, :], lhsT=Wside_sb[s][:],
                rhs=embT16_sb[:, s, j * 512:(j + 1) * 512])
            nc.tensor.matmul(
                s_ps[:, j, :], lhsT=Wmh_sb[s][:],
                rhs=AT_sb[s][:].rearrange("m b n -> m (b n)")[
                    :, j * 512:(j + 1) * 512])
        h2 = fin.tile([128, NW], F32, tag="h2")
        nc.vector.tensor_scalar_add(
            h2[:], h_ps[:].rearrange("p j n -> p (j n)"),
            scalar1=bcols[:, s:s + 1])
        u_sb = fin.tile([128, NW], BF16, tag="u")
        nc.vector.tensor_mul(
            u_sb[:], h2[:], s_ps[:].rearrange("p j n -> p (j n)"))

        # ---- t1^T = (u @ W1)^T + b1 ----
        t1_ps = psw.tile([128, 2, 512], F32, tag="mm1")
        for j in range(2):
            nc.tensor.matmul(t1_ps[:, j, :], lhsT=W1_sb[s][:],
                             rhs=u_sb[:, j * 512:(j + 1) * 512])
        t1T = fin.tile([128, NW], BF16, tag="t1T")
        nc.vector.tensor_scalar_add(
            t1T[:], t1_ps[:].rearrange("p j n -> p (j n)"),
            scalar1=bcols[:, 2 + s:3 + s])

        # ---- LayerNorm stats in node orientation ----
        t1w = psb.tile([128, NW], BF16, tag="tr2")
        for b in range(NB):
            nc.tensor.transpose(
                t1w[:, b * 128:(b + 1) * 128],
                t1T[:, b * 128:(b + 1) * 128], identb16[:])
        t1n = fin.tile([128, NB, 128], BF16, tag="t1n")
        nc.scalar.activation(
            t1n[:].rearrange("p b d -> p (b d)"), t1w[:],
            func=mybir.ActivationFunctionType.Copy)
        sum_t = fin.tile([128, NB], F32, tag="sum")
        nc.vector.tensor_reduce(sum_t[:], t1n[:], axis=mybir.AxisListType.X,
                                op=mybir.AluOpType.add)
        sq = fin.tile([128, NB, 128], BF16, tag="sq")
        nc.scalar.activation(sq[:], t1n[:],
                             func=mybir.ActivationFunctionType.Square)
        ssq = fin.tile([128, NB], F32, tag="ssq")
        nc.vector.tensor_reduce(ssq[:], sq[:], axis=mybir.AxisListType.X,
                                op=mybir.AluOpType.add)
        mu = fin.tile([128, NB], F32, tag="mu")
        nc.vector.tensor_scalar_mul(mu[:], in0=sum_t[:], scalar1=1.0 / D)
        var = fin.tile([128, NB], F32, tag="var")
        nc.vector.tensor_scalar_mul(var[:], in0=ssq[:], scalar1=1.0 / D)
        m2 = fin.tile([128, NB], F32, tag="m2")
        nc.vector.tensor_mul(m2[:], mu[:], mu[:])
        nc.vector.tensor_sub(var[:], var[:], m2[:])
        rstd = fin.tile([128, NB], F32, tag="rstd")
        nc.scalar.activation(rstd[:], var[:],
                             func=mybir.ActivationFunctionType.Sqrt,
                             bias=eps[:], scale=1.0)
        nc.vector.reciprocal(rstd[:], rstd[:])
        that = fin.tile([128, NB, 128], BF16, tag="that")
        for b in range(NB):
            nc.vector.tensor_scalar(
                that[:, b, :], in0=t1n[:, b, :],
                scalar1=mu[:, b:b + 1], scalar2=rstd[:, b:b + 1],
                op0=mybir.AluOpType.subtract, op1=mybir.AluOpType.mult)

        # ---- t2^T = (that @ W2g)^T ; out^T = t2^T + embT + b2p ----
        tt_w = psb.tile([128, NW], BF16, tag="tr2")
        for b in range(NB):
            nc.tensor.transpose(
                tt_w[:, b * 128:(b + 1) * 128], that[:, b, :], identb16[:])
        tT = fin.tile([128, NW], BF16, tag="tT")
        nc.scalar.activation(tT[:], tt_w[:],
                             func=mybir.ActivationFunctionType.Copy)
        t2_ps = psw.tile([128, 2, 512], F32, tag="mm2")
        for j in range(2):
            nc.tensor.matmul(t2_ps[:, j, :], lhsT=W2g_sb[s][:],
                             rhs=tT[:, j * 512:(j + 1) * 512])
        ot = fin.tile([128, NW], F32, tag="ot")
        nc.vector.tensor_add(
            ot[:], t2_ps[:].rearrange("p j n -> p (j n)"),
            embTb2_sb[:, s, :])
        nc.sync.dma_start(out_d.ap(), ot[:])


def assemble(results, perms, cfg):
    NSH = cfg.NSH
    row = np.empty((cfg.N, cfg.D), np.float32)
    col = np.empty((cfg.N, cfg.D), np.float32)
    for c, r in enumerate(results):
        ord_s2d, ord_d2s = perms[c]
        col[c * NSH + ord_s2d] = r["colo"].T
        row[c * NSH + ord_d2s] = r["rowo"].T
    return row, col


# ---------------- graded entry point ----------------

_CACHE = {}


def kernel(**inputs):
    cfg = Cfg()
    in_maps, sched, TOT, perms = host_prep(inputs, cfg)
    key = (sched, TOT)
    if key not in _CACHE:
        _CACHE[key] = build_kernel(cfg, sched, TOT)
    nc = _CACHE[key]
    from concourse.bass_utils import run_bass_kernel_spmd
    res = run_bass_kernel_spmd(nc, in_maps, core_ids=list(range(cfg.C)))
    return assemble(res.results, perms, cfg)
